# revision 2
# baseline (speedup 1.0000x reference)
"""Trainium2 Bass kernel: scatter rows of input_ into a zero-initialized
[output_size, D] bf16 buffer: out[indices[i], :] = input_[i, :] (last
occurrence wins for duplicate indices).

Strategy (8 NeuronCores):
  - Output row-sharded by index range: core k owns rows [k*SHARD, (k+1)*SHARD).
  - Host routing: dedup indices last-wins, bucket rows by (core, 64Ki-row
    region, row parity), pack into fixed-size chunks in the SBUF wrap layout
    dma_scatter_add expects, build wrap-16 replicated int16 index tiles.
  - Device per core: preload all index tiles once, then pipeline
    [load data chunk -> dma_scatter_add into the pre-zeroed output shard].
    dma_scatter_add's CCE add == set because every target row is written
    exactly once onto zeros; padding rows carry zero data and a valid
    in-range index of an unwritten block so their adds are no-ops.
"""

import os
import sys

sys.path.insert(0, "/opt/trn_rl_repo")
os.environ.setdefault("JAX_PLATFORMS", "axon")

import numpy as np
import ml_dtypes

from concourse import bacc, mybir
from concourse.bass import AP
from concourse import bass_utils

N_CORES = 8
REGION_ROWS = 65536  # int16 block index addresses 32768 blocks of 2 rows
CH_CAP = 7936  # per-call index cap: tx ring needs 2*CH/16+1 < 1024 descs
NB = 4  # SBUF data double buffering depth


_prog_cache = {}


def build_program(CH, n_chunks, D, shard_alloc, nsplit, repeats=1):
    """Build the per-core Bass program. Inputs:
      rows [n_chunks*CH, D] bf16 -- chunked rows in wrap-128 layout
      idxw [128, n_chunks*(CH//16)] int16 -- wrap-16 x8-replicated block idx
    Output: out [shard_alloc, D] bf16 (donated pre-zeroed)."""
    key = (CH, n_chunks, D, shard_alloc, nsplit, repeats)
    if key in _prog_cache:
        return _prog_cache[key]
    SL = CH // 128
    CW = CH // 16
    nc = bacc.Bacc(None)
    rows_t = nc.dram_tensor(
        "rows", [n_chunks * CH, D], mybir.dt.bfloat16, kind="ExternalInput"
    )
    idxw_t = nc.dram_tensor(
        "idxw", [128, n_chunks * CW], mybir.dt.int16, kind="ExternalInput"
    )
    out_t = nc.dram_tensor(
        "out", [shard_alloc, D], mybir.dt.bfloat16, kind="ExternalOutput"
    )

    with (
        nc.semaphore("load_sem") as load_sem,
        nc.semaphore("scat_sem") as scat_sem,
    ):
        data_sb = [
            nc.ctx.enter_context(
                nc.sbuf_tensor(f"data{b}", [128, SL * D], mybir.dt.bfloat16)
            )
            for b in range(NB)
        ]
        idx_sb = nc.ctx.enter_context(
            nc.sbuf_tensor("idxs", [128, n_chunks * CW], mybir.dt.int16)
        )

        with nc.Block() as block:

            @block.sync
            def _(sync):
                # one-shot preload of every chunk's index tile
                sync.dma_start(
                    out=AP(idx_sb, 0, [[n_chunks * CW, 128], [1, n_chunks * CW]]),
                    in_=AP(idxw_t, 0, [[n_chunks * CW, 128], [1, n_chunks * CW]]),
                ).then_inc(load_sem, 16)
                t = 0
                for _r in range(repeats):
                    for tc in range(n_chunks):
                        b = t % NB
                        if t >= NB:
                            sync.wait_ge(scat_sem, 16 * (t - NB + 1))
                        sync.dma_start(
                            out=AP(data_sb[b], 0, [[SL * D, 128], [1, SL * D]]),
                            in_=AP(
                                rows_t, tc * CH * D, [[SL * D, 128], [1, SL * D]]
                            ),
                        ).then_inc(load_sem, 16)
                        t += 1

            @block.gpsimd
            def _(g):
                t = 0
                for _r in range(repeats):
                    for tc in range(n_chunks):
                        b = t % NB
                        bucket = tc // nsplit
                        region, par = bucket // 2, bucket % 2
                        g.wait_ge(load_sem, 16 * (t + 2))
                        g.dma_scatter_add(
                            AP(
                                out_t,
                                (region * REGION_ROWS + par) * D,
                                [[2 * D, REGION_ROWS // 2], [1, D]],
                            ),
                            AP(data_sb[b], 0, [[SL * D, 128], [D, SL], [1, D]]),
                            AP(idx_sb, tc * CW, [[n_chunks * CW, 128], [1, CW]]),
                            CH,
                            CH,
                            D,
                            elem_step=2 * D,
                        ).then_inc(scat_sem, 16)
                        t += 1
                g.wait_ge(scat_sem, 16 * t)

    nc.finalize()
    _prog_cache[key] = nc
    return nc


def host_prep(rows, idx, OUT):
    """Dedup + route + pack. Returns (in_maps, geom) where geom =
    (CH, n_chunks, D, shard_alloc, nsplit, SHARD)."""
    N, D = rows.shape
    SHARD = (OUT + N_CORES - 1) // N_CORES

    # ---- host routing ----
    inv = np.full(OUT, -1, dtype=np.int64)
    inv[idx] = np.arange(N)  # last occurrence wins
    win = np.flatnonzero(inv >= 0)  # sorted output rows that get written
    src = inv[win]

    core = win // SHARD
    local = win - core * SHARD
    region = local // REGION_ROWS
    rr = local - region * REGION_ROWS
    par = rr & 1
    blk = (rr >> 1).astype(np.int16)
    n_region = (SHARD + REGION_ROWS - 1) // REGION_ROWS
    n_bucket = n_region * 2
    bucket = region * 2 + par

    key = core * n_bucket + bucket
    counts = np.bincount(key, minlength=N_CORES * n_bucket).reshape(N_CORES, n_bucket)
    maxb = int(counts.max())
    nsplit = max(1, -(-maxb // CH_CAP))
    CH = max(128, min(CH_CAP, ((-(-maxb // nsplit)) + 127) // 128 * 128))
    n_chunks = n_bucket * nsplit
    SL = CH // 128

    order = np.argsort(key, kind="stable")
    k_sorted = key[order]
    blk_sorted = blk[order]
    src_sorted = src[order]
    starts = np.zeros(N_CORES * n_bucket + 1, np.int64)
    np.cumsum(counts.ravel(), out=starts[1:])
    posin = np.arange(len(win)) - starts[k_sorted]  # position within bucket
    bucket_in_core = k_sorted - (k_sorted // n_bucket) * n_bucket
    chunk_in_core = bucket_in_core * nsplit + posin // CH
    posc = posin % CH  # position within chunk
    wrap = (posc % 128) * SL + posc // 128  # wrap layout within chunk

    # Padding rows carry zero data, but their CCE add still does an HBM
    # read-modify-write: a pad colliding with a real row's address can lose
    # the real update (RMW race), and same-address descriptors serialize.
    # So pads target DISTINCT UNWRITTEN blocks of their bucket's region
    # half -- those must be zero anyway, so +0 is harmless.
    NBLK_R = REGION_ROWS // 2
    in_maps = []
    for c in range(N_CORES):
        sel = slice(starts[c * n_bucket], starts[(c + 1) * n_bucket])
        rows_packed = np.zeros((n_chunks * CH, D), dtype=ml_dtypes.bfloat16)
        idx16 = np.empty((n_chunks, CH), dtype=np.int16)
        for b in range(n_bucket):
            s0, s1 = starts[c * n_bucket + b], starts[c * n_bucket + b + 1]
            occ = np.zeros(NBLK_R, dtype=bool)
            occ[blk_sorted[s0:s1].astype(np.int64)] = True
            un = np.flatnonzero(~occ)
            if len(un) == 0:
                un = np.arange(NBLK_R)
            fill = un[np.arange(nsplit * CH) % len(un)].astype(np.int16)
            idx16[b * nsplit : (b + 1) * nsplit] = fill.reshape(nsplit, CH)
        ci = chunk_in_core[sel]
        rows_packed[ci * CH + wrap[sel]] = rows[src_sorted[sel]]
        idx16[ci, posc[sel]] = blk_sorted[sel]
        # wrap-16 layout + replicate x8 to 128 partitions, then lay out as
        # [128, n_chunks*CW] so the whole thing preloads in one DMA
        iw = idx16.reshape(n_chunks, CH // 16, 16).transpose(2, 0, 1)  # [16, nc, CW]
        iwf = np.ascontiguousarray(
            np.broadcast_to(iw[None], (8, 16, n_chunks, CH // 16))
        ).reshape(128, n_chunks * (CH // 16))
        in_maps.append({"rows": rows_packed, "idxw": iwf})

    shard_alloc = n_region * REGION_ROWS
    return in_maps, (CH, n_chunks, D, shard_alloc, nsplit, SHARD)


LAST_PREP = None


def kernel(input_, indices, output_size, n_tpc):
    global LAST_PREP
    rows = np.asarray(input_)
    in_dtype = rows.dtype
    if rows.dtype != ml_dtypes.bfloat16:
        rows = rows.astype(ml_dtypes.bfloat16)
    idx = np.asarray(indices).astype(np.int64)
    OUT = int(output_size)

    in_maps, geom = host_prep(rows, idx, OUT)
    LAST_PREP = (in_maps, geom)
    CH, n_chunks, D, shard_alloc, nsplit, SHARD = geom
    nc = build_program(CH, n_chunks, D, shard_alloc, nsplit)
    res = bass_utils.run_bass_kernel_spmd(nc, in_maps, core_ids=list(range(N_CORES)))

    out_full = np.concatenate(
        [r["out"][:SHARD] for r in res.results], axis=0
    )[:OUT]
    return np.ascontiguousarray(out_full.astype(in_dtype))


# revision 3
# speedup vs baseline: 1.1569x; 1.1569x over previous
"""Trainium2 Bass kernel: scatter rows of input_ into a zero-initialized
[output_size, D] bf16 buffer: out[indices[i], :] = input_[i, :] (last
occurrence wins for duplicate indices).

Strategy (8 NeuronCores):
  - Output row-sharded by index range: core k owns rows [k*SHARD, (k+1)*SHARD).
  - Host routing: dedup indices last-wins, bucket rows by (core, 64Ki-row
    region, row parity), pack into fixed-size chunks in the SBUF wrap layout
    dma_scatter_add expects, build wrap-16 replicated int16 index tiles.
  - Device per core: preload all index tiles once, then pipeline
    [load data chunk -> dma_scatter_add into the pre-zeroed output shard].
    dma_scatter_add's CCE add == set because every target row is written
    exactly once onto zeros; padding rows carry zero data and a valid
    in-range index of an unwritten block so their adds are no-ops.
"""

import os
import sys

sys.path.insert(0, "/opt/trn_rl_repo")
os.environ.setdefault("JAX_PLATFORMS", "axon")

import numpy as np
import ml_dtypes

from concourse import bacc, mybir
from concourse.bass import AP
from concourse import bass_utils

N_CORES = 8
REGION_ROWS = 65536  # int16 block index addresses 32768 blocks of 2 rows
CH_CAP = 7936  # per-call index cap: tx ring needs 2*CH/16+1 < 1024 descs
NQ = 4  # SWDGE queues: queue q's descriptors are generated by Q7 core pair q,
#         so spreading scatters over 4 queues parallelizes descriptor gen 4x
NB = 8  # SBUF data buffers (multiple of NQ so per-queue FIFO completion is safe)


_prog_cache = {}


def build_program(CH, n_chunks, D, shard_alloc, nsplit, repeats=1):
    """Build the per-core Bass program. Inputs:
      rows [n_chunks*CH, D] bf16 -- chunked rows in wrap-128 layout
      idxw [128, n_chunks*(CH//16)] int16 -- wrap-16 x8-replicated block idx
    Output: out [shard_alloc, D] bf16 (donated pre-zeroed)."""
    key = (CH, n_chunks, D, shard_alloc, nsplit, repeats)
    if key in _prog_cache:
        return _prog_cache[key]
    SL = CH // 128
    CW = CH // 16
    nc = bacc.Bacc(None)
    rows_t = nc.dram_tensor(
        "rows", [n_chunks * CH, D], mybir.dt.bfloat16, kind="ExternalInput"
    )
    idxw_t = nc.dram_tensor(
        "idxw", [128, n_chunks * CW], mybir.dt.int16, kind="ExternalInput"
    )
    out_t = nc.dram_tensor(
        "out", [shard_alloc, D], mybir.dt.bfloat16, kind="ExternalOutput"
    )

    with (
        nc.semaphore("load_sem") as load_sem,
        nc.semaphore("scat_sem") as scat_sem,
    ):
        data_sb = [
            nc.ctx.enter_context(
                nc.sbuf_tensor(f"data{b}", [128, SL * D], mybir.dt.bfloat16)
            )
            for b in range(NB)
        ]
        idx_sb = nc.ctx.enter_context(
            nc.sbuf_tensor("idxs", [128, n_chunks * CW], mybir.dt.int16)
        )

        with nc.Block() as block:

            @block.sync
            def _(sync):
                # one-shot preload of every chunk's index tile
                sync.dma_start(
                    out=AP(idx_sb, 0, [[n_chunks * CW, 128], [1, n_chunks * CW]]),
                    in_=AP(idxw_t, 0, [[n_chunks * CW, 128], [1, n_chunks * CW]]),
                ).then_inc(load_sem, 16)
                t = 0
                for _r in range(repeats):
                    for tc in range(n_chunks):
                        b = t % NB
                        if t >= NB:
                            sync.wait_ge(scat_sem, 16 * (t - NB + 1))
                        sync.dma_start(
                            out=AP(data_sb[b], 0, [[SL * D, 128], [1, SL * D]]),
                            in_=AP(
                                rows_t, tc * CH * D, [[SL * D, 128], [1, SL * D]]
                            ),
                        ).then_inc(load_sem, 16)
                        t += 1

            @block.gpsimd
            def _(g):
                t = 0
                for _r in range(repeats):
                    for tc in range(n_chunks):
                        b = t % NB
                        bucket = tc // nsplit
                        region, par = bucket // 2, bucket % 2
                        g.wait_ge(load_sem, 16 * (t + 2))
                        g.dma_scatter_add(
                            AP(
                                out_t,
                                (region * REGION_ROWS + par) * D,
                                [[2 * D, REGION_ROWS // 2], [1, D]],
                            ),
                            AP(data_sb[b], 0, [[SL * D, 128], [D, SL], [1, D]]),
                            AP(idx_sb, tc * CW, [[n_chunks * CW, 128], [1, CW]]),
                            CH,
                            CH,
                            D,
                            elem_step=2 * D,
                        ).then_inc(scat_sem, 16)
                        t += 1
                g.wait_ge(scat_sem, 16 * t)

    nc.finalize()
    _prog_cache[key] = nc
    return nc


def host_prep(rows, idx, OUT):
    """Dedup + route + pack. Returns (in_maps, geom) where geom =
    (CH, n_chunks, D, shard_alloc, nsplit, SHARD)."""
    N, D = rows.shape
    SHARD = (OUT + N_CORES - 1) // N_CORES

    # ---- host routing ----
    inv = np.full(OUT, -1, dtype=np.int64)
    inv[idx] = np.arange(N)  # last occurrence wins
    win = np.flatnonzero(inv >= 0)  # sorted output rows that get written
    src = inv[win]

    core = win // SHARD
    local = win - core * SHARD
    region = local // REGION_ROWS
    rr = local - region * REGION_ROWS
    par = rr & 1
    blk = (rr >> 1).astype(np.int16)
    n_region = (SHARD + REGION_ROWS - 1) // REGION_ROWS
    n_bucket = n_region * 2
    bucket = region * 2 + par

    key = core * n_bucket + bucket
    counts = np.bincount(key, minlength=N_CORES * n_bucket).reshape(N_CORES, n_bucket)
    maxb = int(counts.max())
    nsplit = max(1, -(-maxb // CH_CAP))
    CH = max(128, min(CH_CAP, ((-(-maxb // nsplit)) + 127) // 128 * 128))
    n_chunks = n_bucket * nsplit
    SL = CH // 128

    order = np.argsort(key, kind="stable")
    k_sorted = key[order]
    blk_sorted = blk[order]
    src_sorted = src[order]
    starts = np.zeros(N_CORES * n_bucket + 1, np.int64)
    np.cumsum(counts.ravel(), out=starts[1:])
    posin = np.arange(len(win)) - starts[k_sorted]  # position within bucket
    bucket_in_core = k_sorted - (k_sorted // n_bucket) * n_bucket
    chunk_in_core = bucket_in_core * nsplit + posin // CH
    posc = posin % CH  # position within chunk
    wrap = (posc % 128) * SL + posc // 128  # wrap layout within chunk

    # Padding rows carry zero data, but their CCE add still does an HBM
    # read-modify-write: a pad colliding with a real row's address can lose
    # the real update (RMW race), and same-address descriptors serialize.
    # So pads target DISTINCT UNWRITTEN blocks of their bucket's region
    # half -- those must be zero anyway, so +0 is harmless.
    NBLK_R = REGION_ROWS // 2
    in_maps = []
    for c in range(N_CORES):
        sel = slice(starts[c * n_bucket], starts[(c + 1) * n_bucket])
        rows_packed = np.zeros((n_chunks * CH, D), dtype=ml_dtypes.bfloat16)
        idx16 = np.empty((n_chunks, CH), dtype=np.int16)
        for b in range(n_bucket):
            s0, s1 = starts[c * n_bucket + b], starts[c * n_bucket + b + 1]
            occ = np.zeros(NBLK_R, dtype=bool)
            occ[blk_sorted[s0:s1].astype(np.int64)] = True
            un = np.flatnonzero(~occ)
            if len(un) == 0:
                un = np.arange(NBLK_R)
            fill = un[np.arange(nsplit * CH) % len(un)].astype(np.int16)
            idx16[b * nsplit : (b + 1) * nsplit] = fill.reshape(nsplit, CH)
        ci = chunk_in_core[sel]
        rows_packed[ci * CH + wrap[sel]] = rows[src_sorted[sel]]
        idx16[ci, posc[sel]] = blk_sorted[sel]
        # wrap-16 layout + replicate x8 to 128 partitions, then lay out as
        # [128, n_chunks*CW] so the whole thing preloads in one DMA
        iw = idx16.reshape(n_chunks, CH // 16, 16).transpose(2, 0, 1)  # [16, nc, CW]
        iwf = np.ascontiguousarray(
            np.broadcast_to(iw[None], (8, 16, n_chunks, CH // 16))
        ).reshape(128, n_chunks * (CH // 16))
        in_maps.append({"rows": rows_packed, "idxw": iwf})

    shard_alloc = n_region * REGION_ROWS
    return in_maps, (CH, n_chunks, D, shard_alloc, nsplit, SHARD)


LAST_PREP = None


def kernel(input_, indices, output_size, n_tpc):
    global LAST_PREP
    rows = np.asarray(input_)
    in_dtype = rows.dtype
    if rows.dtype != ml_dtypes.bfloat16:
        rows = rows.astype(ml_dtypes.bfloat16)
    idx = np.asarray(indices).astype(np.int64)
    OUT = int(output_size)

    in_maps, geom = host_prep(rows, idx, OUT)
    LAST_PREP = (in_maps, geom)
    CH, n_chunks, D, shard_alloc, nsplit, SHARD = geom
    nc = build_program(CH, n_chunks, D, shard_alloc, nsplit)
    res = bass_utils.run_bass_kernel_spmd(nc, in_maps, core_ids=list(range(N_CORES)))

    out_full = np.concatenate(
        [r["out"][:SHARD] for r in res.results], axis=0
    )[:OUT]
    return np.ascontiguousarray(out_full.astype(in_dtype))
